# revision 4
# baseline (speedup 1.0000x reference)
"""Trainium2 Bass kernel for nn_MixedLipMlp (soft-MoE MLP with Lipschitz gate).

Strategy: data-parallel over batch B=4096 across 8 NeuronCores (512 rows each,
expert weights + gate replicated). Activations live feature-major (features on
partitions, batch on the free dim) for layers 0/1; layer 2 stays feature-major
too (out [96, 512] = experts o-major) and the coefficient mix is ONE one-hot
partition-sum matmul, so the output DMA is 12 large packets instead of 128
tiny ones (the v1 batch-major mix cost ~4.3us of DMA tail).

v3 changes vs the 92.7us v1 baseline:
  - layer 2 feature-major + one-hot mix matmul (kills the coeffB machinery,
    the per-bt prod/reduce, and the slow [128, 48] output DMA).
  - constant-bias fast path: the reference initializes expert biases to a
    constant (b = 0.01); when all-equal, coeff @ b == b (softmax sums to 1),
    so the bias folds into the ELU epilogue as a per-partition ACT bias and
    the 8 bias matmuls + bpack DMA disappear. General-bias fallback keeps the
    v1 bias-matmul path.
  - k-major (slab-major) h-passes in layers 0/1: the DVE produces the
    coeff-scaled rhs tiles just-in-time, one k-slab ahead of the PE, instead
    of needing all slabs of an expert up front.
  - mid-layer ELU = exp on ACT + relu on ACT + min on DVE; the final k-group
    of each layer runs m-outer so PSUM banks close staggered and the ELU
    pipeline overlaps the remaining matmuls.
  - DMA: critical inputs (gate weights, z, c) first on sync/scalar; the big
    weight streams are held behind a tiny cross-queue dependency on z so they
    don't contend with the gate-critical transfers, then stream need-ordered
    (w0 before w1 halves, wz1/w2 between).
"""

import os
import sys

if "/opt/trn_rl_repo" not in sys.path:
    sys.path.insert(0, "/opt/trn_rl_repo")

# recover cleanly if a previous process left the NeuronCores wedged
os.environ.setdefault("NEURON_RT_RESET_CORES", "1")

import numpy as np

# Problem dimensions (hardcoded; must match the grader's setup_inputs()).
B = 4096
NCORES = 8
BS = B // NCORES  # 512 batch rows per core = matmul free dim
LATENT = 64
INPUT_SIZE = 256
IN_DIM = LATENT + INPUT_SIZE  # 320
HIDDEN = 512
ACTIONS = 12
E = 8
GATE_H = 128
INTER = HIDDEN + LATENT  # 576

NK0 = 2   # layer0: c has 256 rows = 2 k-slabs
NK12 = 4  # layers1,2: h has 512 rows = 4 k-slabs
N_M = HIDDEN // 128  # 4 output m-tiles for layers 0/1
NA = E * ACTIONS  # 96: layer-2 outputs packed o-major (col = o*8+e)

TRACE = False
LAST_EXEC_NS = None
LAST_RESULTS = None


def _build_nc(bias_const):
    import concourse.mybir as mybir
    from concourse import bacc
    from concourse.tile import TileContext

    dt = mybir.dt
    F32 = dt.float32
    F16 = dt.float16
    AF = mybir.ActivationFunctionType
    OP = mybir.AluOpType

    nc = bacc.Bacc("TRN2", target_bir_lowering=False)

    # ---- DRAM I/O ------------------------------------------------------
    # gatepack cols: gw0a(0:128,rows<64) gw0b(128:256) gw0c(256:384)
    #   gw1(384:512) gw2(512:520); all Lipschitz-folded on the host
    d_gate = nc.dram_tensor("gatepack", [128, 520], F16, kind="ExternalInput")
    # cpack cols: sel8(0:1024) sel96(1024:1120) on rows<8;
    #   sel12(1120:1132) on rows<96; gate biases gb0 gb1 gb2 b0c b1c
    #   (1132:1137): gb2 on rows<8, b0c/b1c only used on the const-bias path
    d_cp = nc.dram_tensor("cpack", [128, 1137], F16, kind="ExternalInput")
    # per-core inputs: xinA = c0(0:512) c1(512:1024); xinB = zT
    d_xinA = nc.dram_tensor("xinA", [128, 1024], F16, kind="ExternalInput")
    d_xinB = nc.dram_tensor("xinB", [LATENT, 512], F16, kind="ExternalInput")
    d_wz0 = nc.dram_tensor("wz0", [128, E // 2 * HIDDEN], F16,
                           kind="ExternalInput")
    d_wz1 = nc.dram_tensor("wz1", [128, E // 2 * HIDDEN], F16,
                           kind="ExternalInput")
    # w2pack cols: w2h slabs (0:384) all 128 rows; w2z+b2 (384:480) rows<65
    d_w2 = nc.dram_tensor("w2pack", [128, 480], F16, kind="ExternalInput")
    d_w0h = nc.dram_tensor("w0hcat", [128, E * NK0 * HIDDEN], F16,
                           kind="ExternalInput")
    d_w1h = nc.dram_tensor("w1hcat", [128, E * NK12 * HIDDEN], F16,
                           kind="ExternalInput")
    d_bp = nc.dram_tensor("bpack", [E, 2 * HIDDEN], F16, kind="ExternalInput")
    d_out = nc.dram_tensor("outF", [ACTIONS, BS], F32, kind="ExternalOutput")

    mm = nc.tensor.matmul
    SL0 = NK0 * HIDDEN    # 1024 cols per l0 expert slab block
    SL1 = NK12 * HIDDEN   # 2048 cols per l1 expert slab block

    with TileContext(nc) as tc:
        from contextlib import ExitStack

        with ExitStack() as ctx:
            pers = ctx.enter_context(tc.tile_pool(name="pers", bufs=1))
            sca = ctx.enter_context(tc.tile_pool(name="sca", bufs=10))
            etmp = ctx.enter_context(tc.tile_pool(name="etmp", bufs=4))

            # ---- DMA: 3 queues, need-ordered -----------------------------
            # critical gate data first: gate+xinA on sync, xinB+cpack on
            # scalar (scalar stays short so the ACT stream frees up before
            # the first gate exp)
            gate = pers.tile([128, 520], F16, tag="gate")
            nc.sync.dma_start(out=gate, in_=d_gate[:, :])
            xinA = pers.tile([128, 1024], F16, tag="xinA")
            nc.sync.dma_start(out=xinA, in_=d_xinA[:, :])
            xinB = pers.tile([LATENT, 512], F16, tag="xinB")
            nc.scalar.dma_start(out=xinB, in_=d_xinB[:, :])
            cp = pers.tile([128, 1137], F16, tag="cp")
            nc.scalar.dma_start(out=cp, in_=d_cp[:, :])

            # weight streams wait for the critical inputs (cross-queue read
            # gates), then stream in need-order.
            w0hcat = pers.tile([128, E * SL0], F16, tag="w0hcat")
            w1hcat = pers.tile([128, E * SL1], F16, tag="w1hcat")

            dumb = pers.tile([1, 8], F16, tag="dumb")
            nc.gpsimd.tensor_copy(out=dumb[0:1, 0:4], in_=xinB[0:1, 0:4])
            wz0 = pers.tile([128, E // 2 * HIDDEN], F16, tag="wz0")
            nc.gpsimd.dma_start(out=wz0, in_=d_wz0[:, :])
            nc.gpsimd.dma_start(out=w0hcat, in_=d_w0h[:, :])
            wz1 = pers.tile([128, E // 2 * HIDDEN], F16, tag="wz1")
            nc.gpsimd.dma_start(out=wz1, in_=d_wz1[:, :])
            # l1 h slabs: experts 0-5 on sync (after xinA), 6-7 + w2 + bias
            # pack on gpsimd
            nc.sync.dma_start(out=w1hcat[:, 0:6 * SL1],
                              in_=d_w1h[:, 0:6 * SL1])
            nc.gpsimd.dma_start(out=w1hcat[:, 6 * SL1:],
                                in_=d_w1h[:, 6 * SL1:])
            w2p = pers.tile([128, 480], F16, tag="w2p")
            nc.gpsimd.dma_start(out=w2p, in_=d_w2[:, :])
            if not bias_const:
                bp = pers.tile([E, 2 * HIDDEN], F16, tag="bp")
                nc.gpsimd.dma_start(out=bp, in_=d_bp[:, :])
                b0sb = bp[:, 0:HIDDEN]
                b1sb = bp[:, HIDDEN:]

            gw0t = [gate[0:64, 0:128], gate[:, 128:256], gate[:, 256:384]]
            gw1t = gate[:, 384:512]
            gw2t = gate[:, 512:520]
            sel8 = cp[0:E, 0:1024]
            sel96 = cp[0:E, 1024:1120]
            sel12 = cp[0:NA, 1120:1132]
            # biases as f32 (DVE tensor_scalar add requires an f32 scalar AP)
            gbf = pers.tile([128, 5], F32, tag="gbf")
            nc.scalar.activation(out=gbf, in_=cp[:, 1132:1137], func=AF.Copy)
            gb0 = gbf[:, 0:1]
            gb1 = gbf[:, 1:2]
            gb2 = gbf[0:E, 2:3]
            b0c = gbf[:, 3:4]
            b1c = gbf[:, 4:5]
            xc = [xinA[:, 0:512], xinA[:, 512:1024]]
            w0h = [w0hcat[:, e * SL0:(e + 1) * SL0] for e in range(E)]
            w1h = [w1hcat[:, e * SL1:(e + 1) * SL1] for e in range(E)]
            w2h = [w2p[:, k * NA:(k + 1) * NA] for k in range(NK12)]
            w2z = w2p[0:LATENT + 1, 384:480]

            # ---- constants + on-device z expansion -----------------------
            ones_blk = pers.tile([128, 128], F16, tag="ones_blk")
            nc.vector.memset(ones_blk, 1.0)
            warm_rhs = pers.tile([128, BS], F16, tag="warm_rhs")
            nc.vector.memset(warm_rhs, 0.0)
            # z duplicated into both row halves for the zsf scalings; ones
            # row appended for the l2 bias
            xz2 = pers.tile([128, BS], F16, tag="xz2")
            nc.vector.tensor_copy(out=xz2[0:LATENT, :], in_=xinB)
            nc.vector.tensor_copy(out=xz2[LATENT:128, :], in_=xinB)
            xzo = pers.tile([LATENT + 1, BS], F16, tag="xzo")
            nc.vector.tensor_copy(out=xzo[0:LATENT, :], in_=xinB)
            nc.vector.memset(xzo[LATENT:LATENT + 1, :], 1.0)

            # ---- gate + softmax + coefficient broadcasts -----------------
            # the whole chain is column-split into two 256-wide halves so
            # the serial latency (mm -> elu -> mm -> ... -> coeff) pipelines
            HB = BS // 2
            halves = [slice(0, HB), slice(HB, BS)]
            with tc.tile_pool(name="ps_g", bufs=2, space="PSUM") as ps_g, \
                 tc.tile_pool(name="ps_bc", bufs=2, space="PSUM") as ps_bc:

                # trip the PE activity monitor before the gate chain
                for _ in range(4):
                    pw = ps_bc.tile([128, BS], F32, tag="bc",
                                    name=f"warm{nc.next_id()}")
                    mm(pw, ones_blk, warm_rhs, start=True, stop=True)

                def gate_elu_h(ps, bias, out, sl):
                    # elu(y) = min(exp(y)-1, relu(y)); exp on ACT and relu
                    # on DVE run concurrently (latency-critical chain)
                    ex = etmp.tile([ps.shape[0], HB], F16, tag="elu_exp",
                                   name=f"gex{nc.next_id()}")
                    nc.scalar.activation(out=ex, in_=ps, func=AF.Exp,
                                         bias=bias)
                    rl = etmp.tile([ps.shape[0], HB], F16, tag="elu_relu",
                                   name=f"grl{nc.next_id()}")
                    nc.vector.tensor_scalar(rl, ps, bias, 0.0, OP.add, OP.max)
                    nc.vector.scalar_tensor_tensor(
                        out=out[:, sl], in0=ex, scalar=1.0, in1=rl,
                        op0=OP.subtract, op1=OP.min,
                    )

                h0g = pers.tile([GATE_H, BS], F16, tag="h0g")
                h1g = pers.tile([GATE_H, BS], F16, tag="h1g")
                expl = pers.tile([E, BS], F16, tag="expl")
                bcR = pers.tile([128, BS], F32, tag="bcR")
                coeffT = pers.tile([E, BS], F16, tag="coeffT")
                rhs0 = [xinB, xc[0], xc[1]]
                psg0, psg1, pslg, pssum = [], [], [], []
                for hi, sl in enumerate(halves):
                    p = ps_g.tile([GATE_H, HB], F32, tag="g", name=f"psg0{hi}")
                    for k in range(3):
                        mm(p, gw0t[k], rhs0[k][:, sl],
                           start=(k == 0), stop=(k == 2))
                    psg0.append(p)
                for hi, sl in enumerate(halves):
                    gate_elu_h(psg0[hi], gb0, h0g, sl)
                for hi, sl in enumerate(halves):
                    p = ps_g.tile([GATE_H, HB], F32, tag="g", name=f"psg1{hi}")
                    mm(p, gw1t, h0g[:, sl], start=True, stop=True)
                    psg1.append(p)
                for hi, sl in enumerate(halves):
                    gate_elu_h(psg1[hi], gb1, h1g, sl)
                for hi, sl in enumerate(halves):
                    p = ps_g.tile([E, HB], F32, tag="lg", name=f"pslg{hi}",
                                  bufs=2)
                    mm(p, gw2t, h1g[:, sl], start=True, stop=True)
                    pslg.append(p)
                # softmax over the 8 expert partitions (logits bounded by the
                # lip constraint, no max subtraction needed)
                for hi, sl in enumerate(halves):
                    nc.scalar.activation(out=expl[:, sl], in_=pslg[hi],
                                         func=AF.Exp, bias=gb2)
                    p = ps_bc.tile([128, HB], F32, tag="sum", name=f"pss{hi}",
                                   bufs=1)
                    mm(p, ones_blk[:E, :], expl[:, sl], start=True, stop=True)
                    pssum.append(p)
                for hi, sl in enumerate(halves):
                    nc.vector.reciprocal_approx_fast(out=bcR[:, sl],
                                                     in_=pssum[hi])
                    nc.vector.tensor_mul(coeffT[:, sl], expl[:, sl],
                                         bcR[:E, sl])

                # broadcast each normalized coeff row to all 128 partitions
                bcE = []
                for e in range(E):
                    pb = ps_bc.tile([128, BS], F32, tag="bc", name=f"pbc{e}",
                                    bufs=2)
                    mm(pb, sel8[:, 128 * e: 128 * (e + 1)], coeffT,
                       start=True, stop=True)
                    t = pers.tile([128, BS], F16, tag=f"bcE{e}")
                    nc.scalar.activation(out=t, in_=pb, func=AF.Copy)
                    bcE.append(t)

                # coeff broadcast to the 96 (o-major) l2 output partitions:
                # cX[o*8+e, b] = coeff[e, b]
                pcx = ps_bc.tile([128, BS], F32, tag="bc", name="pcx")
                mm(pcx[0:NA, :], sel96, coeffT, start=True, stop=True)
                cX = pers.tile([NA, BS], F16, tag="cX")
                nc.scalar.activation(out=cX, in_=pcx[0:NA, :], func=AF.Copy)

            # coeff-scaled z per expert (z duplicated in both 64-row halves
            # so an expert can ride either PE row group); shared by l0 and l1
            zsf = []
            for e in range(E):
                t = pers.tile([128, BS], F16, tag=f"zsf{e}")
                nc.vector.tensor_mul(t, xz2, bcE[e])
                zsf.append(t)
            # l0 scaled c inputs, k-major so the PE can consume slab 0 of
            # every expert while slab 1 is still being produced
            cs = [[None] * E for _ in range(NK0)]
            for ki in range(NK0):
                for e in range(E):
                    t = sca.tile([128, BS], F16, tag="s", name=f"c{ki}_{e}")
                    nc.vector.tensor_mul(t, xc[ki], bcE[e])
                    cs[ki][e] = t

            def z_pass(wz, psl, start):
                # row-paired z matmuls: two experts concurrently in disjoint
                # PE row groups; T1/T2 packing swaps experts between groups
                # so each group covers all 4 m-slices (top -> banks {0,1},
                # bottom -> banks {2,3}).
                for p in range(E // 2):
                    for t_ in range(2):
                        base = p * HIDDEN + t_ * 256
                        etop = 2 * p + t_
                        ebot = 2 * p + 1 - t_
                        st = start and p == 0 and t_ == 0
                        for mi in range(2):
                            mm(psl[mi],
                               wz[:LATENT, base + 128 * mi: base + 128 * (mi + 1)],
                               zsf[etop][:LATENT, :],
                               start=st, stop=False)
                            mm(psl[2 + mi],
                               wz[LATENT:, base + 128 * mi: base + 128 * (mi + 1)],
                               zsf[ebot][LATENT:, :],
                               start=st, stop=False)

            def moe_elu(psl_m, bias, out_tag):
                # elu(y + b) = min(exp(y+b)-1, relu(y+b)); exp and relu both
                # on ACT (throughput path; DVE is loaded with the scalings),
                # min on DVE
                ex = etmp.tile([128, BS], F16, tag="elu_exp",
                               name=f"mex{nc.next_id()}")
                nc.scalar.activation(out=ex, in_=psl_m, func=AF.Exp,
                                     bias=bias)
                rl = etmp.tile([128, BS], F16, tag="elu_relu",
                               name=f"mrl{nc.next_id()}")
                if bias is None:
                    nc.scalar.activation(out=rl, in_=psl_m, func=AF.Relu)
                else:
                    nc.scalar.activation(out=rl, in_=psl_m, func=AF.Relu,
                                         bias=bias)
                h = pers.tile([128, BS], F16, tag=out_tag, name=out_tag)
                nc.vector.scalar_tensor_tensor(
                    out=h, in0=ex, scalar=1.0, in1=rl,
                    op0=OP.subtract, op1=OP.min,
                )
                return h

            def h_pass(wh, hs_tiles, nk, psl, bsb, bias, htag):
                # k-major: all experts' slab-ki matmuls before slab ki+1, so
                # each scaled rhs tile is consumed right after the DVE makes
                # it. The last k-group runs m-outer so banks close staggered
                # and the ELUs pipeline.
                for ki in range(nk - 1):
                    for e in range(E):
                        for m in range(N_M):
                            mm(psl[m], wh[e][:, ki * HIDDEN + 128 * m:
                                             ki * HIDDEN + 128 * (m + 1)],
                               hs_tiles[ki][e], start=False, stop=False)
                ki = nk - 1
                hts = []
                for m in range(N_M):
                    for e in range(E):
                        last = e == E - 1
                        if last and bsb is not None:
                            mm(psl[m], wh[e][:, ki * HIDDEN + 128 * m:
                                             ki * HIDDEN + 128 * (m + 1)],
                               hs_tiles[ki][e], start=False, stop=False)
                            mm(psl[m], bsb[:, 128 * m: 128 * (m + 1)], coeffT,
                               start=False, stop=True)
                        else:
                            mm(psl[m], wh[e][:, ki * HIDDEN + 128 * m:
                                             ki * HIDDEN + 128 * (m + 1)],
                               hs_tiles[ki][e], start=False, stop=last)
                    hts.append(moe_elu(psl[m], bias, f"{htag}{m}"))
                return hts

            # ---- MoE layers 0+1 share all 8 PSUM banks -------------------
            acc_ctx = tc.tile_pool(name="ps_acc", bufs=8, space="PSUM")
            ps_acc = acc_ctx.__enter__()
            ps_l0 = [ps_acc.tile([128, BS], F32, tag="acc", name=f"psl0_{m}")
                     for m in range(N_M)]
            z_pass(wz0, ps_l0, start=True)
            if bias_const:
                h0m = h_pass(w0h, cs, NK0, ps_l0, None, b0c, "h0m")
            else:
                h0m = h_pass(w0h, cs, NK0, ps_l0, b0sb, None, "h0m")

            # l1 scaled h inputs, k-major
            hs1 = [[None] * E for _ in range(NK12)]
            for ki in range(NK12):
                for e in range(E):
                    t = sca.tile([128, BS], F16, tag="s", name=f"h{ki}_{e}")
                    nc.vector.tensor_mul(t, h0m[ki], bcE[e])
                    hs1[ki][e] = t

            ps_l1 = [ps_acc.tile([128, BS], F32, tag="acc", name=f"psl1_{m}")
                     for m in range(N_M)]
            z_pass(wz1, ps_l1, start=True)
            if bias_const:
                h1m = h_pass(w1h, hs1, NK12, ps_l1, None, b1c, "h1m")
            else:
                h1m = h_pass(w1h, hs1, NK12, ps_l1, b1sb, None, "h1m")
            acc_ctx.__exit__(None, None, None)

            # ---- MoE layer 2, feature-major: y96[o*8+e, b], then the mix
            # is one elementwise multiply + one one-hot partition-sum matmul
            with tc.tile_pool(name="ps_l2", bufs=1, space="PSUM") as ps_l2:
                y96 = ps_l2.tile([NA, BS], F32, tag="y96", name="y96")
                mm(y96, w2z, xzo, start=True, stop=False)
                for k in range(NK12):
                    mm(y96, w2h[k], h1m[k], start=False, stop=(k == NK12 - 1))
                ymix = pers.tile([NA, BS], F16, tag="ymix")
                nc.vector.tensor_mul(ymix, y96, cX)
                out12 = ps_l2.tile([ACTIONS, BS], F32, tag="o12", name="o12")
                mm(out12, sel12, ymix, start=True, stop=True)
                acto = pers.tile([ACTIONS, BS], F32, tag="acto")
                nc.scalar.activation(out=acto, in_=out12, func=AF.Copy)
                nc.sync.dma_start(out=d_out[:, :], in_=acto)

    nc.finalize()
    return nc


_nc_cache = {}


def _get_nc(bias_const):
    if bias_const not in _nc_cache:
        _nc_cache[bias_const] = _build_nc(bias_const)
    return _nc_cache[bias_const]


def _patch_hook_errors():
    # exceptions inside the neuronx-cc hook are swallowed by the PJRT
    # plugin ("CallFunctionObjArgs: error condition"); print them here
    from concourse import bass2jax

    orig = bass2jax.neuronx_cc_hook
    if getattr(orig, "_err_patched", False):
        return

    def wrapped(*a, **k):
        import traceback

        try:
            return orig(*a, **k)
        except BaseException as e:
            print(getattr(e, "output", ""), file=sys.stderr)
            traceback.print_exc()
            raise

    wrapped._err_patched = True
    bass2jax.neuronx_cc_hook = wrapped


def _pack_z_pairs(w):
    # (E, in, out) -> (128, E/2*out). For each expert pair p, two tiles of
    # (128, out/2): T1 = [top: even expert, first half of m-slices;
    # bottom: odd expert, second half], T2 = the swap — so the top PE row
    # group only ever produces the first half of output banks and the bottom
    # the second half, while both experts cover all output columns.
    z = w[:, :LATENT, :]
    out = z.shape[2]
    h = out // 2
    blk = np.empty((128, E // 2, 2, h), np.float32)
    for p in range(E // 2):
        blk[:LATENT, p, 0] = z[2 * p, :, :h]
        blk[LATENT:, p, 0] = z[2 * p + 1, :, h:]
        blk[:LATENT, p, 1] = z[2 * p + 1, :, :h]
        blk[LATENT:, p, 1] = z[2 * p, :, h:]
    return blk.reshape(128, -1)


def _lip_fold(gw, gc):
    # LipschitzLinear: rows of W scaled so row-wise L1 norm <= softplus(c);
    # depends only on the weights, so fold it on the host.
    lipc = np.logaddexp(0.0, np.float64(gc.reshape(())))
    scale = np.minimum(lipc / np.abs(np.float64(gw)).sum(1), 1.0)
    return (np.float64(gw) * scale[:, None]).astype(np.float32)


def _pack_weights(f, bias_const):
    c = np.ascontiguousarray
    f16 = np.float16

    gate = np.zeros((128, 520), np.float32)
    gw0 = _lip_fold(f["gw0"], f["gc0"]).T  # [320, 128]
    gate[0:64, 0:128] = gw0[0:64]
    gate[:, 128:256] = gw0[64:192]
    gate[:, 256:384] = gw0[192:320]
    gate[:, 384:512] = _lip_fold(f["gw1"], f["gc1"]).T
    gate[:, 512:520] = _lip_fold(f["gw2"], f["gc2"]).T

    cp = np.zeros((128, 1137), np.float32)
    for e in range(E):
        cp[e, 128 * e: 128 * (e + 1)] = 1.0                # sel8
        cp[e, 1024 + np.arange(ACTIONS) * E + e] = 1.0     # sel96, o-major
    for p in range(NA):
        cp[p, 1120 + p // E] = 1.0                         # sel12
    cp[:, 1132] = f["gb0"]
    cp[:, 1133] = f["gb1"]
    cp[0:E, 1134] = f["gb2"]
    if bias_const:
        cp[:, 1135] = f["b0"].flat[0]
        cp[:, 1136] = f["b1"].flat[0]

    w2 = f["w2"]  # (E, 576, 12); l2 outputs packed o-major: col = o*8+e
    w2p = np.zeros((128, 480), np.float32)
    w2p[:, 0:384] = (w2[:, LATENT:, :].reshape(E, NK12, 128, ACTIONS)
                     .transpose(2, 1, 3, 0).reshape(128, -1))
    w2p[0:LATENT, 384:480] = (w2[:, :LATENT, :].transpose(1, 2, 0)
                              .reshape(LATENT, -1))
    w2p[LATENT, 384:480] = f["b2"].T.reshape(-1)  # bias rides the ones row

    out = {
        "gatepack": c(gate.astype(f16)),
        "cpack": c(cp.astype(f16)),
        "wz0": c(_pack_z_pairs(f["w0"]).astype(f16)),
        "wz1": c(_pack_z_pairs(f["w1"]).astype(f16)),
        "w2pack": c(w2p.astype(f16)),
        "w0hcat": c(f["w0"][:, LATENT:, :].reshape(E, NK0, 128, HIDDEN)
                    .transpose(2, 0, 1, 3).reshape(128, -1).astype(f16)),
        "w1hcat": c(f["w1"][:, LATENT:, :].reshape(E, NK12, 128, HIDDEN)
                    .transpose(2, 0, 1, 3).reshape(128, -1).astype(f16)),
        "bpack": c(np.concatenate([f["b0"], f["b1"]], axis=1).astype(f16)),
    }
    return out


def kernel(**inputs):
    global LAST_EXEC_NS, LAST_RESULTS
    from concourse import bass_utils

    _patch_hook_errors()

    f = {k: np.ascontiguousarray(np.asarray(v, dtype=np.float32))
         for k, v in inputs.items()}

    bias_const = bool(
        np.all(f["b0"] == f["b0"].flat[0]) and np.all(f["b1"] == f["b1"].flat[0])
    )

    shared = _pack_weights(f, bias_const)
    in_maps = []
    for ci in range(NCORES):
        sl = slice(ci * BS, (ci + 1) * BS)
        m = dict(shared)
        m["xinA"] = np.ascontiguousarray(
            f["c"][sl].T.reshape(2, 128, 512).transpose(1, 0, 2)
            .reshape(128, 1024).astype(np.float16))
        m["xinB"] = np.ascontiguousarray(f["z"][sl].T.astype(np.float16))
        in_maps.append(m)

    nc = _get_nc(bias_const)
    res = bass_utils.run_bass_kernel_spmd(
        nc, in_maps, list(range(NCORES)), trace=TRACE
    )
    LAST_EXEC_NS = res.exec_time_ns
    LAST_RESULTS = res
    out = np.concatenate(
        [np.asarray(res.results[ci]["outF"]).T for ci in range(NCORES)],
        axis=0,
    )
    return np.ascontiguousarray(out)


# revision 9
# speedup vs baseline: 1.0475x; 1.0475x over previous
"""Trainium2 Bass kernel for nn_MixedLipMlp (soft-MoE MLP with Lipschitz gate).

Strategy: data-parallel over batch B=4096 across 8 NeuronCores (512 rows each,
expert weights + gate replicated). Activations live feature-major (features on
partitions, batch on the free dim) for layers 0/1; layer 2 stays feature-major
too (out [96, 512] = experts o-major) and the coefficient mix is ONE one-hot
partition-sum matmul, so the output DMA is 12 large packets instead of 128
tiny ones (the v1 batch-major mix cost ~4.3us of DMA tail).

v3 changes vs the 92.7us v1 baseline:
  - layer 2 feature-major + one-hot mix matmul (kills the coeffB machinery,
    the per-bt prod/reduce, and the slow [128, 48] output DMA).
  - constant-bias fast path: the reference initializes expert biases to a
    constant (b = 0.01); when all-equal, coeff @ b == b (softmax sums to 1),
    so the bias folds into the ELU epilogue as a per-partition ACT bias and
    the 8 bias matmuls + bpack DMA disappear. General-bias fallback keeps the
    v1 bias-matmul path.
  - k-major (slab-major) h-passes in layers 0/1: the DVE produces the
    coeff-scaled rhs tiles just-in-time, one k-slab ahead of the PE, instead
    of needing all slabs of an expert up front.
  - mid-layer ELU = exp on ACT + relu on ACT + min on DVE; the final k-group
    of each layer runs m-outer so PSUM banks close staggered and the ELU
    pipeline overlaps the remaining matmuls.
  - DMA: critical inputs (gate weights, z, c) first on sync/scalar; the big
    weight streams are held behind a tiny cross-queue dependency on z so they
    don't contend with the gate-critical transfers, then stream need-ordered
    (w0 before w1 halves, wz1/w2 between).
"""

import os
import sys

if "/opt/trn_rl_repo" not in sys.path:
    sys.path.insert(0, "/opt/trn_rl_repo")

# recover cleanly if a previous process left the NeuronCores wedged
os.environ.setdefault("NEURON_RT_RESET_CORES", "1")

import numpy as np

# Problem dimensions (hardcoded; must match the grader's setup_inputs()).
B = 4096
NCORES = 8
BS = B // NCORES  # 512 batch rows per core = matmul free dim
LATENT = 64
INPUT_SIZE = 256
IN_DIM = LATENT + INPUT_SIZE  # 320
HIDDEN = 512
ACTIONS = 12
E = 8
GATE_H = 128
INTER = HIDDEN + LATENT  # 576

NK0 = 2   # layer0: c has 256 rows = 2 k-slabs
NK12 = 4  # layers1,2: h has 512 rows = 4 k-slabs
N_M = HIDDEN // 128  # 4 output m-tiles for layers 0/1
NA = E * ACTIONS  # 96: layer-2 outputs packed o-major (col = o*8+e)

TRACE = False
LAST_EXEC_NS = None
LAST_RESULTS = None


def _build_nc(bias_const):
    import concourse.mybir as mybir
    from concourse import bacc
    from concourse.tile import TileContext

    dt = mybir.dt
    F32 = dt.float32
    F16 = dt.float16
    AF = mybir.ActivationFunctionType
    OP = mybir.AluOpType

    nc = bacc.Bacc("TRN2", target_bir_lowering=False)

    # ---- DRAM I/O ------------------------------------------------------
    # gatepack cols: gw0a(0:128,rows<64) gw0b(128:256) gw0c(256:384)
    #   gw1(384:512) gw2(512:520); all Lipschitz-folded on the host
    d_gate = nc.dram_tensor("gatepack", [128, 520], F16, kind="ExternalInput")
    # selpack cols: sel8(0:1024) sel96(1024:1120)
    d_selp = nc.dram_tensor("selpack", [E, 1120], F16, kind="ExternalInput")
    d_sel12 = nc.dram_tensor("sel12p", [NA, ACTIONS], F16,
                             kind="ExternalInput")
    # gate biases gb0 gb1 gb2 + (const-bias path) b0c b1c
    d_gbp = nc.dram_tensor("gbpack", [128, 5], F32, kind="ExternalInput")
    # per-core inputs: xinA = c0(0:512) c1(512:1024); xinB = zT
    d_xinA = nc.dram_tensor("xinA", [128, 1024], F16, kind="ExternalInput")
    d_xinB = nc.dram_tensor("xinB", [LATENT, 512], F16, kind="ExternalInput")
    d_wz0 = nc.dram_tensor("wz0", [128, E // 2 * HIDDEN], F16,
                           kind="ExternalInput")
    d_wz1 = nc.dram_tensor("wz1", [128, E // 2 * HIDDEN], F16,
                           kind="ExternalInput")
    # w2pack cols: w2h slabs (0:384) all 128 rows; w2z+b2 (384:480) rows<65
    d_w2 = nc.dram_tensor("w2pack", [128, 480], F16, kind="ExternalInput")
    d_w0h = nc.dram_tensor("w0hcat", [128, E * NK0 * HIDDEN], F16,
                           kind="ExternalInput")
    d_w1h = nc.dram_tensor("w1hcat", [128, E * NK12 * HIDDEN], F16,
                           kind="ExternalInput")
    d_bp = nc.dram_tensor("bpack", [E, 2 * HIDDEN], F16, kind="ExternalInput")
    d_out = nc.dram_tensor("outF", [ACTIONS, BS], F32, kind="ExternalOutput")

    mm = nc.tensor.matmul
    SL0 = NK0 * HIDDEN    # 1024 cols per l0 expert slab block
    SL1 = NK12 * HIDDEN   # 2048 cols per l1 expert slab block

    with TileContext(nc) as tc:
        from contextlib import ExitStack

        with ExitStack() as ctx:
            pers = ctx.enter_context(tc.tile_pool(name="pers", bufs=1))
            sca = ctx.enter_context(tc.tile_pool(name="sca", bufs=10))
            etmp = ctx.enter_context(tc.tile_pool(name="etmp", bufs=4))

            # ---- DMA: 3 queues, need-ordered -----------------------------
            # critical gate data first: gate+xinA on sync; the tiny
            # bias/selector packs on scalar (so the ACT stream frees up
            # before the first gate exp)
            gate = pers.tile([128, 520], F16, tag="gate")
            nc.sync.dma_start(out=gate, in_=d_gate[:, :])
            xinA = pers.tile([128, 1024], F16, tag="xinA")
            nc.sync.dma_start(out=xinA, in_=d_xinA[:, :])
            gbp = pers.tile([128, 5], F32, tag="gbp")
            nc.scalar.dma_start(out=gbp, in_=d_gbp[:, :])
            xinB = pers.tile([LATENT, 512], F16, tag="xinB")
            nc.scalar.dma_start(out=xinB, in_=d_xinB[:, :])
            selp = pers.tile([E, 1120], F16, tag="selp")
            nc.scalar.dma_start(out=selp, in_=d_selp[:, :])
            sel12 = pers.tile([NA, ACTIONS], F16, tag="sel12")
            nc.scalar.dma_start(out=sel12, in_=d_sel12[:, :])

            # Weight streams on sync+gpsimd, chunked (a single multi-MB
            # transfer hogs the DMA hardware and starves later queue
            # entries) and gated behind the critical inputs by tiny
            # cross-queue reads.
            w0hcat = pers.tile([128, E * SL0], F16, tag="w0hcat")
            w1hcat = pers.tile([128, E * SL1], F16, tag="w1hcat")

            dumb = pers.tile([1, 8], F16, tag="dumb")
            nc.gpsimd.tensor_copy(out=dumb[0:1, 0:4], in_=xinB[0:1, 0:4])
            wz0 = pers.tile([128, E // 2 * HIDDEN], F16, tag="wz0")
            nc.gpsimd.dma_start(out=wz0, in_=d_wz0[:, :])
            for c0 in range(4):
                nc.gpsimd.dma_start(
                    out=w0hcat[:, 2 * c0 * SL0:2 * (c0 + 1) * SL0],
                    in_=d_w0h[:, 2 * c0 * SL0:2 * (c0 + 1) * SL0])
            wz1 = pers.tile([128, E // 2 * HIDDEN], F16, tag="wz1")
            nc.gpsimd.dma_start(out=wz1, in_=d_wz1[:, :])
            nc.gpsimd.dma_start(out=w1hcat[:, 6 * SL1:],
                                in_=d_w1h[:, 6 * SL1:])
            w2p = pers.tile([128, 480], F16, tag="w2p")
            nc.gpsimd.dma_start(out=w2p, in_=d_w2[:, :])
            if not bias_const:
                bp = pers.tile([E, 2 * HIDDEN], F16, tag="bp")
                nc.gpsimd.dma_start(out=bp, in_=d_bp[:, :])
                b0sb = bp[:, 0:HIDDEN]
                b1sb = bp[:, HIDDEN:]

            # l1 h slabs for experts 0-5 on sync, held behind the first w0
            # chunk (w0 is needed ~15us earlier than w1)
            dumb2 = pers.tile([1, 8], F16, tag="dumb2")
            nc.sync.dma_start(out=dumb2[0:1, 0:4], in_=w0hcat[0:1, 0:4])
            for c1 in range(3):
                nc.sync.dma_start(
                    out=w1hcat[:, 2 * c1 * SL1:2 * (c1 + 1) * SL1],
                    in_=d_w1h[:, 2 * c1 * SL1:2 * (c1 + 1) * SL1])

            gw0t = [gate[0:64, 0:128], gate[:, 128:256], gate[:, 256:384]]
            gw1t = gate[:, 384:512]
            gw2t = gate[:, 512:520]
            sel8 = selp[:, 0:1024]
            sel96 = selp[:, 1024:1120]
            gb0 = gbp[:, 0:1]
            gb1 = gbp[:, 1:2]
            gb2 = gbp[0:E, 2:3]
            b0c = gbp[:, 3:4]
            b1c = gbp[:, 4:5]
            xc = [xinA[:, 0:512], xinA[:, 512:1024]]
            w0h = [w0hcat[:, e * SL0:(e + 1) * SL0] for e in range(E)]
            w1h = [w1hcat[:, e * SL1:(e + 1) * SL1] for e in range(E)]
            w2h = [w2p[:, k * NA:(k + 1) * NA] for k in range(NK12)]
            w2z = w2p[0:LATENT + 1, 384:480]

            # ---- constants + on-device z expansion -----------------------
            ones_blk = pers.tile([128, 128], F16, tag="ones_blk")
            nc.vector.memset(ones_blk, 1.0)
            warm_rhs = pers.tile([128, BS], F16, tag="warm_rhs")
            nc.vector.memset(warm_rhs, 0.0)
            # z duplicated into both row halves for the zsf scalings; ones
            # row appended for the l2 bias
            xz2 = pers.tile([128, BS], F16, tag="xz2")
            nc.vector.tensor_copy(out=xz2[0:LATENT, :], in_=xinB)
            nc.vector.tensor_copy(out=xz2[LATENT:128, :], in_=xinB)
            xzo = pers.tile([LATENT + 1, BS], F16, tag="xzo")
            nc.vector.tensor_copy(out=xzo[0:LATENT, :], in_=xinB)
            nc.vector.memset(xzo[LATENT:LATENT + 1, :], 1.0)

            # ---- gate + softmax + coefficient broadcasts -----------------
            # the whole chain is column-split into two 256-wide halves so
            # the serial latency (mm -> elu -> mm -> ... -> coeff) pipelines
            HB = BS // 2
            halves = [slice(0, HB), slice(HB, BS)]
            with tc.tile_pool(name="ps_g", bufs=2, space="PSUM") as ps_g, \
                 tc.tile_pool(name="ps_bc", bufs=2, space="PSUM") as ps_bc:

                # trip the PE activity monitor before the gate chain
                for _ in range(4):
                    pw = ps_bc.tile([128, BS], F32, tag="bc",
                                    name=f"warm{nc.next_id()}")
                    mm(pw, ones_blk, warm_rhs, start=True, stop=True)

                def gate_elu_h(ps, bias, out, sl):
                    # elu(y) = min(exp(y)-1, relu(y)); exp on ACT and relu
                    # on DVE run concurrently (latency-critical chain)
                    ex = etmp.tile([ps.shape[0], HB], F16, tag="elu_exp",
                                   name=f"gex{nc.next_id()}")
                    nc.scalar.activation(out=ex, in_=ps, func=AF.Exp,
                                         bias=bias)
                    rl = etmp.tile([ps.shape[0], HB], F16, tag="elu_relu",
                                   name=f"grl{nc.next_id()}")
                    nc.vector.tensor_scalar(rl, ps, bias, 0.0, OP.add, OP.max)
                    nc.vector.scalar_tensor_tensor(
                        out=out[:, sl], in0=ex, scalar=1.0, in1=rl,
                        op0=OP.subtract, op1=OP.min,
                    )

                h0g = pers.tile([GATE_H, BS], F16, tag="h0g")
                h1g = pers.tile([GATE_H, BS], F16, tag="h1g")
                expl = pers.tile([E, BS], F16, tag="expl")
                bcR = pers.tile([128, BS], F32, tag="bcR")
                coeffT = pers.tile([E, BS], F16, tag="coeffT")
                rhs0 = [xinB, xc[0], xc[1]]
                psg0, psg1, pslg, pssum = [], [], [], []
                for hi, sl in enumerate(halves):
                    p = ps_g.tile([GATE_H, HB], F32, tag="g", name=f"psg0{hi}")
                    for k in range(3):
                        mm(p, gw0t[k], rhs0[k][:, sl],
                           start=(k == 0), stop=(k == 2))
                    psg0.append(p)
                for hi, sl in enumerate(halves):
                    gate_elu_h(psg0[hi], gb0, h0g, sl)
                for hi, sl in enumerate(halves):
                    p = ps_g.tile([GATE_H, HB], F32, tag="g", name=f"psg1{hi}")
                    mm(p, gw1t, h0g[:, sl], start=True, stop=True)
                    psg1.append(p)
                for hi, sl in enumerate(halves):
                    gate_elu_h(psg1[hi], gb1, h1g, sl)
                for hi, sl in enumerate(halves):
                    p = ps_g.tile([E, HB], F32, tag="lg", name=f"pslg{hi}",
                                  bufs=2)
                    mm(p, gw2t, h1g[:, sl], start=True, stop=True)
                    pslg.append(p)
                # softmax over the 8 expert partitions (logits bounded by the
                # lip constraint, no max subtraction needed)
                for hi, sl in enumerate(halves):
                    nc.scalar.activation(out=expl[:, sl], in_=pslg[hi],
                                         func=AF.Exp, bias=gb2)
                    p = ps_bc.tile([128, HB], F32, tag="sum", name=f"pss{hi}",
                                   bufs=1)
                    mm(p, ones_blk[:E, :], expl[:, sl], start=True, stop=True)
                    pssum.append(p)
                for hi, sl in enumerate(halves):
                    nc.vector.reciprocal_approx_fast(out=bcR[:, sl],
                                                     in_=pssum[hi])
                    nc.vector.tensor_mul(coeffT[:, sl], expl[:, sl],
                                         bcR[:E, sl])

                # broadcast each normalized coeff row to all 128 partitions
                bcE = []
                for e in range(E):
                    pb = ps_bc.tile([128, BS], F32, tag="bc", name=f"pbc{e}",
                                    bufs=2)
                    mm(pb, sel8[:, 128 * e: 128 * (e + 1)], coeffT,
                       start=True, stop=True)
                    t = pers.tile([128, BS], F16, tag=f"bcE{e}")
                    nc.scalar.activation(out=t, in_=pb, func=AF.Copy)
                    bcE.append(t)

                # coeff broadcast to the 96 (o-major) l2 output partitions:
                # cX[o*8+e, b] = coeff[e, b]
                pcx = ps_bc.tile([128, BS], F32, tag="bc", name="pcx")
                mm(pcx[0:NA, :], sel96, coeffT, start=True, stop=True)
                cX = pers.tile([NA, BS], F16, tag="cX")
                nc.scalar.activation(out=cX, in_=pcx[0:NA, :], func=AF.Copy)

            # coeff-scaled z per expert (z duplicated in both 64-row halves
            # so an expert can ride either PE row group); shared by l0 and l1
            zsf = []
            for e in range(E):
                t = pers.tile([128, BS], F16, tag=f"zsf{e}")
                nc.vector.tensor_mul(t, xz2, bcE[e])
                zsf.append(t)
            # l0 scaled c inputs, k-major so the PE can consume slab 0 of
            # every expert while slab 1 is still being produced
            cs = [[None] * E for _ in range(NK0)]
            for ki in range(NK0):
                for e in range(E):
                    t = sca.tile([128, BS], F16, tag="s", name=f"c{ki}_{e}")
                    nc.vector.tensor_mul(t, xc[ki], bcE[e])
                    cs[ki][e] = t

            def z_pass(wz, psl, start):
                # row-paired z matmuls: two experts concurrently in disjoint
                # PE row groups; T1/T2 packing swaps experts between groups
                # so each group covers all 4 m-slices (top -> banks {0,1},
                # bottom -> banks {2,3}).
                for p in range(E // 2):
                    for t_ in range(2):
                        base = p * HIDDEN + t_ * 256
                        etop = 2 * p + t_
                        ebot = 2 * p + 1 - t_
                        st = start and p == 0 and t_ == 0
                        for mi in range(2):
                            mm(psl[mi],
                               wz[:LATENT, base + 128 * mi: base + 128 * (mi + 1)],
                               zsf[etop][:LATENT, :],
                               start=st, stop=False)
                            mm(psl[2 + mi],
                               wz[LATENT:, base + 128 * mi: base + 128 * (mi + 1)],
                               zsf[ebot][LATENT:, :],
                               start=st, stop=False)

            def moe_elu(psl_m, bias, out_tag):
                # elu(y + b) = min(exp(y+b)-1, relu(y+b)); exp and relu both
                # on ACT (throughput path; DVE is loaded with the scalings),
                # min on DVE
                ex = etmp.tile([128, BS], F16, tag="elu_exp",
                               name=f"mex{nc.next_id()}")
                nc.scalar.activation(out=ex, in_=psl_m, func=AF.Exp,
                                     bias=bias)
                rl = etmp.tile([128, BS], F16, tag="elu_relu",
                               name=f"mrl{nc.next_id()}")
                if bias is None:
                    nc.scalar.activation(out=rl, in_=psl_m, func=AF.Relu)
                else:
                    nc.scalar.activation(out=rl, in_=psl_m, func=AF.Relu,
                                         bias=bias)
                h = pers.tile([128, BS], F16, tag=out_tag, name=out_tag)
                nc.vector.scalar_tensor_tensor(
                    out=h, in0=ex, scalar=1.0, in1=rl,
                    op0=OP.subtract, op1=OP.min,
                )
                return h

            def h_pass(wh, hs_tiles, nk, psl, bsb, bias, htag):
                # k-major: all experts' slab-ki matmuls before slab ki+1, so
                # each scaled rhs tile is consumed right after the DVE makes
                # it. The last k-group runs m-outer so banks close staggered
                # and the ELUs pipeline.
                for ki in range(nk - 1):
                    for e in range(E):
                        for m in range(N_M):
                            mm(psl[m], wh[e][:, ki * HIDDEN + 128 * m:
                                             ki * HIDDEN + 128 * (m + 1)],
                               hs_tiles[ki][e], start=False, stop=False)
                ki = nk - 1
                hts = []
                for m in range(N_M):
                    for e in range(E):
                        last = e == E - 1
                        if last and bsb is not None:
                            mm(psl[m], wh[e][:, ki * HIDDEN + 128 * m:
                                             ki * HIDDEN + 128 * (m + 1)],
                               hs_tiles[ki][e], start=False, stop=False)
                            mm(psl[m], bsb[:, 128 * m: 128 * (m + 1)], coeffT,
                               start=False, stop=True)
                        else:
                            mm(psl[m], wh[e][:, ki * HIDDEN + 128 * m:
                                             ki * HIDDEN + 128 * (m + 1)],
                               hs_tiles[ki][e], start=False, stop=last)
                    hts.append(moe_elu(psl[m], bias, f"{htag}{m}"))
                return hts

            # ---- MoE layers 0+1 share all 8 PSUM banks -------------------
            acc_ctx = tc.tile_pool(name="ps_acc", bufs=8, space="PSUM")
            ps_acc = acc_ctx.__enter__()
            ps_l0 = [ps_acc.tile([128, BS], F32, tag="acc", name=f"psl0_{m}")
                     for m in range(N_M)]
            z_pass(wz0, ps_l0, start=True)
            if bias_const:
                h0m = h_pass(w0h, cs, NK0, ps_l0, None, b0c, "h0m")
            else:
                h0m = h_pass(w0h, cs, NK0, ps_l0, b0sb, None, "h0m")

            # l1 scaled h inputs, k-major
            hs1 = [[None] * E for _ in range(NK12)]
            for ki in range(NK12):
                for e in range(E):
                    t = sca.tile([128, BS], F16, tag="s", name=f"h{ki}_{e}")
                    nc.vector.tensor_mul(t, h0m[ki], bcE[e])
                    hs1[ki][e] = t

            ps_l1 = [ps_acc.tile([128, BS], F32, tag="acc", name=f"psl1_{m}")
                     for m in range(N_M)]
            z_pass(wz1, ps_l1, start=True)
            if bias_const:
                h1m = h_pass(w1h, hs1, NK12, ps_l1, None, b1c, "h1m")
            else:
                h1m = h_pass(w1h, hs1, NK12, ps_l1, b1sb, None, "h1m")
            acc_ctx.__exit__(None, None, None)

            # ---- MoE layer 2, feature-major: y96[o*8+e, b], then the mix
            # is one elementwise multiply + one one-hot partition-sum matmul
            with tc.tile_pool(name="ps_l2", bufs=1, space="PSUM") as ps_l2:
                y96 = ps_l2.tile([NA, BS], F32, tag="y96", name="y96")
                mm(y96, w2z, xzo, start=True, stop=False)
                for k in range(NK12):
                    mm(y96, w2h[k], h1m[k], start=False, stop=(k == NK12 - 1))
                ymix = pers.tile([NA, BS], F16, tag="ymix")
                nc.vector.tensor_mul(ymix, y96, cX)
                out12 = ps_l2.tile([ACTIONS, BS], F32, tag="o12", name="o12")
                mm(out12, sel12, ymix, start=True, stop=True)
                acto = pers.tile([ACTIONS, BS], F32, tag="acto")
                nc.scalar.activation(out=acto, in_=out12, func=AF.Copy)
                nc.sync.dma_start(out=d_out[:, :], in_=acto)

    nc.finalize()
    return nc


_nc_cache = {}


def _get_nc(bias_const):
    if bias_const not in _nc_cache:
        _nc_cache[bias_const] = _build_nc(bias_const)
    return _nc_cache[bias_const]


def _patch_hook_errors():
    # exceptions inside the neuronx-cc hook are swallowed by the PJRT
    # plugin ("CallFunctionObjArgs: error condition"); print them here
    from concourse import bass2jax

    orig = bass2jax.neuronx_cc_hook
    if getattr(orig, "_err_patched", False):
        return

    def wrapped(*a, **k):
        import traceback

        try:
            return orig(*a, **k)
        except BaseException as e:
            print(getattr(e, "output", ""), file=sys.stderr)
            traceback.print_exc()
            raise

    wrapped._err_patched = True
    bass2jax.neuronx_cc_hook = wrapped


def _pack_z_pairs(w):
    # (E, in, out) -> (128, E/2*out). For each expert pair p, two tiles of
    # (128, out/2): T1 = [top: even expert, first half of m-slices;
    # bottom: odd expert, second half], T2 = the swap — so the top PE row
    # group only ever produces the first half of output banks and the bottom
    # the second half, while both experts cover all output columns.
    z = w[:, :LATENT, :]
    out = z.shape[2]
    h = out // 2
    blk = np.empty((128, E // 2, 2, h), np.float32)
    for p in range(E // 2):
        blk[:LATENT, p, 0] = z[2 * p, :, :h]
        blk[LATENT:, p, 0] = z[2 * p + 1, :, h:]
        blk[:LATENT, p, 1] = z[2 * p + 1, :, :h]
        blk[LATENT:, p, 1] = z[2 * p, :, h:]
    return blk.reshape(128, -1)


def _lip_fold(gw, gc):
    # LipschitzLinear: rows of W scaled so row-wise L1 norm <= softplus(c);
    # depends only on the weights, so fold it on the host.
    lipc = np.logaddexp(0.0, np.float64(gc.reshape(())))
    scale = np.minimum(lipc / np.abs(np.float64(gw)).sum(1), 1.0)
    return (np.float64(gw) * scale[:, None]).astype(np.float32)


def _pack_weights(f, bias_const):
    c = np.ascontiguousarray
    f16 = np.float16

    gate = np.zeros((128, 520), np.float32)
    gw0 = _lip_fold(f["gw0"], f["gc0"]).T  # [320, 128]
    gate[0:64, 0:128] = gw0[0:64]
    gate[:, 128:256] = gw0[64:192]
    gate[:, 256:384] = gw0[192:320]
    gate[:, 384:512] = _lip_fold(f["gw1"], f["gc1"]).T
    gate[:, 512:520] = _lip_fold(f["gw2"], f["gc2"]).T

    selp = np.zeros((E, 1120), np.float32)
    for e in range(E):
        selp[e, 128 * e: 128 * (e + 1)] = 1.0              # sel8
        selp[e, 1024 + np.arange(ACTIONS) * E + e] = 1.0   # sel96, o-major
    s12 = np.zeros((NA, ACTIONS), np.float32)
    s12[np.arange(NA), np.arange(NA) // E] = 1.0           # sel12
    gbp = np.zeros((128, 5), np.float32)
    gbp[:, 0] = f["gb0"]
    gbp[:, 1] = f["gb1"]
    gbp[0:E, 2] = f["gb2"]
    if bias_const:
        gbp[:, 3] = f["b0"].flat[0]
        gbp[:, 4] = f["b1"].flat[0]

    w2 = f["w2"]  # (E, 576, 12); l2 outputs packed o-major: col = o*8+e
    w2p = np.zeros((128, 480), np.float32)
    w2p[:, 0:384] = (w2[:, LATENT:, :].reshape(E, NK12, 128, ACTIONS)
                     .transpose(2, 1, 3, 0).reshape(128, -1))
    w2p[0:LATENT, 384:480] = (w2[:, :LATENT, :].transpose(1, 2, 0)
                              .reshape(LATENT, -1))
    w2p[LATENT, 384:480] = f["b2"].T.reshape(-1)  # bias rides the ones row

    out = {
        "gatepack": c(gate.astype(f16)),
        "selpack": c(selp.astype(f16)),
        "sel12p": c(s12.astype(f16)),
        "gbpack": c(gbp),
        "wz0": c(_pack_z_pairs(f["w0"]).astype(f16)),
        "wz1": c(_pack_z_pairs(f["w1"]).astype(f16)),
        "w2pack": c(w2p.astype(f16)),
        "w0hcat": c(f["w0"][:, LATENT:, :].reshape(E, NK0, 128, HIDDEN)
                    .transpose(2, 0, 1, 3).reshape(128, -1).astype(f16)),
        "w1hcat": c(f["w1"][:, LATENT:, :].reshape(E, NK12, 128, HIDDEN)
                    .transpose(2, 0, 1, 3).reshape(128, -1).astype(f16)),
        "bpack": c(np.concatenate([f["b0"], f["b1"]], axis=1).astype(f16)),
    }
    return out


def kernel(**inputs):
    global LAST_EXEC_NS, LAST_RESULTS
    from concourse import bass_utils

    _patch_hook_errors()

    f = {k: np.ascontiguousarray(np.asarray(v, dtype=np.float32))
         for k, v in inputs.items()}

    bias_const = bool(
        np.all(f["b0"] == f["b0"].flat[0]) and np.all(f["b1"] == f["b1"].flat[0])
    )

    shared = _pack_weights(f, bias_const)
    in_maps = []
    for ci in range(NCORES):
        sl = slice(ci * BS, (ci + 1) * BS)
        m = dict(shared)
        m["xinA"] = np.ascontiguousarray(
            f["c"][sl].T.reshape(2, 128, 512).transpose(1, 0, 2)
            .reshape(128, 1024).astype(np.float16))
        m["xinB"] = np.ascontiguousarray(f["z"][sl].T.astype(np.float16))
        in_maps.append(m)

    nc = _get_nc(bias_const)
    res = bass_utils.run_bass_kernel_spmd(
        nc, in_maps, list(range(NCORES)), trace=TRACE
    )
    LAST_EXEC_NS = res.exec_time_ns
    LAST_RESULTS = res
    out = np.concatenate(
        [np.asarray(res.results[ci]["outF"]).T for ci in range(NCORES)],
        axis=0,
    )
    return np.ascontiguousarray(out)
